# revision 5
# baseline (speedup 1.0000x reference)
"""Trainium2 Bass kernel for nn_Attention_28930899706081 (sparse_attention).

Reference computation:
  k1 = l2norm_c(Wqk @ fmap1), k2 = l2norm_c(Wqk @ fmap2), q = l2norm_c(Wqk @ dmap)
  sim_i = q^T k_i per batch  -> [b, n, n] with n = h*w = 4096
  attn_i = softmax(sim_i, axis=-1)[:, None]  -> [b, 1, n, n]
  returns (attn1, attn2)

Sharding: 8 cores; core i handles batch b = i//4 and query-row block r = i%4
(1024 of 4096 rows). Each core computes the full normalized K for its batch
(recompute instead of collectives) and its row block of both sims + softmax.

ScalarE (ACT) is the bottleneck: the 8.4M softmax exps per core stream at
1 elem/cycle/lane @1.2GHz (~64us minimum). Everything else must stay off
ScalarE and overlap:
  - squares for the column norms run on VectorE (psum*psum -> bf16)
  - only Abs_reciprocal_sqrt (9 chunks) + Exp run on ACT, grouped by table
    set (all ars strictly before the first exp; a dummy exp right after the
    last ars preloads the exp table set during the phase-A ramp)
  - row sums come from the ACT activation accumulator fused with exp
  - attn normalize mul on VectorE, output in fp16 (better quantization than
    bf16; attn values ~3e-4 sit mid fp16 normal range)
|sim| <= 1 because q/k are unit vectors, so softmax needs no max subtraction.
"""

import numpy as np
import ml_dtypes

B, C, H, W, D = 2, 256, 64, 64, 128
N = H * W  # 4096
QBLK = N // 4  # 1024 query rows per core
N_CORES = 8

_cached = {}


def _build():
    import concourse.mybir as mybir
    import concourse.tile as tile
    from concourse.tile_rust import add_dep_helper
    from concourse import bacc
    from contextlib import ExitStack

    f32 = mybir.dt.float32
    f16 = mybir.dt.float16
    bf16 = mybir.dt.bfloat16
    AF = mybir.ActivationFunctionType

    nc = bacc.Bacc(
        "TRN2",
        target_bir_lowering=False,
        debug=False,
        enable_asserts=False,
        num_devices=N_CORES,
    )

    f1_ext = nc.dram_tensor("f1", [C, N], bf16, kind="ExternalInput").ap()
    f2_ext = nc.dram_tensor("f2", [C, N], bf16, kind="ExternalInput").ap()
    xq_ext = nc.dram_tensor("xq", [C, QBLK], bf16, kind="ExternalInput").ap()
    wqkT_ext = nc.dram_tensor("wqkT", [C, D], bf16, kind="ExternalInput").ap()
    out_ext = nc.dram_tensor("out", [2, QBLK, N], f16, kind="ExternalOutput").ap()

    XCH = 1024  # phase A chunk (proj psum [128,1024] = 2 banks)
    CH = 2048  # phase B sim/exp chunk ([128,2048] = 4 banks)

    with tile.TileContext(nc) as tc, ExitStack() as ctx:
        consts = ctx.enter_context(tc.tile_pool(name="consts", bufs=1))
        xin = ctx.enter_context(tc.tile_pool(name="xin", bufs=12))
        ysq_pool = ctx.enter_context(tc.tile_pool(name="ysq", bufs=3))
        rk_pool = ctx.enter_context(tc.tile_pool(name="rk", bufs=3))
        kn_pool = ctx.enter_context(tc.tile_pool(name="kn", bufs=1))
        e_pool = ctx.enter_context(tc.tile_pool(name="epool", bufs=8))
        attn_pool = ctx.enter_context(tc.tile_pool(name="attn", bufs=4))
        stat_pool = ctx.enter_context(tc.tile_pool(name="stat", bufs=4))

        # constants
        wqkT_sb = [
            consts.tile([128, D], bf16, tag=f"wqkT{k}", name=f"wqkT{k}")
            for k in range(2)
        ]
        nc.gpsimd.dma_start(out=wqkT_sb[0][:], in_=wqkT_ext[0:128, :])
        nc.gpsimd.dma_start(out=wqkT_sb[1][:], in_=wqkT_ext[128:256, :])
        ones_sb = consts.tile([128, 128], bf16, tag="ones", name="ones")
        nc.vector.memset(ones_sb[:], 1.0)
        # prime the ACT table set: a dummy Abs_reciprocal_sqrt loads
        # abs_reciprocal_sqrt_and_small up front so phase A ars's don't pay
        # the table switch mid-pipeline.
        warm = consts.tile([128, 1], f32, tag="warm", name="warm")
        nc.scalar.activation(out=warm[:], in_=ones_sb[:, 0:1], func=AF.Abs_reciprocal_sqrt)

        last_ars = None

        with tc.tile_pool(name="proj_psum", bufs=2, space="PSUM") as proj_psum, \
             tc.tile_pool(name="n2_psum", bufs=2, space="PSUM") as n2_psum:

            def norm_chunk(x_lo, x_hi, xn, h0, ncols):
                """project + column-l2-normalize one [128, ncols] chunk.

                TensorE: 2 proj matmuls (c-halves) + 1 ones matmul (partition
                reduction, broadcast). DVE: square (psum*psum) + normalize mul.
                ACT: one Abs_reciprocal_sqrt.
                """
                nonlocal last_ars
                ps = proj_psum.tile([128, XCH], f32, tag="proj", name="pps")
                for c in range(ncols // 512):
                    sl = slice(c * 512, (c + 1) * 512)
                    nc.tensor.matmul(
                        ps[:, sl], wqkT_sb[0][:], x_lo[:, sl],
                        start=True, stop=False,
                    )
                    nc.tensor.matmul(
                        ps[:, sl], wqkT_sb[1][:], x_hi[:, sl],
                        start=False, stop=True,
                    )
                y_bf = ysq_pool.tile([128, XCH], bf16, tag="ybf", name="y_bf")
                nc.vector.tensor_copy(y_bf[:, 0:ncols], ps[:, 0:ncols])
                ysq = ysq_pool.tile([128, XCH], bf16, tag="ysq", name="ysq")
                nc.vector.tensor_mul(ysq[:, 0:ncols], y_bf[:, 0:ncols], y_bf[:, 0:ncols])
                nps = n2_psum.tile([128, XCH], f32, tag="n2", name="nps")
                for c in range(ncols // 512):
                    sl = slice(c * 512, (c + 1) * 512)
                    nc.tensor.matmul(
                        nps[:, sl], ones_sb[:], ysq[:, sl],
                        start=True, stop=True,
                    )
                rk = rk_pool.tile([128, XCH], f16, tag="rk", name="rk")
                last_ars = nc.scalar.activation(
                    out=rk[:, 0:ncols], in_=nps[:, 0:ncols],
                    func=AF.Abs_reciprocal_sqrt,
                )
                nc.vector.tensor_mul(
                    xn[:, h0 : h0 + ncols], y_bf[:, 0:ncols], rk[:, 0:ncols]
                )

            def phase_a(x_ext, ncols, tagbase):
                xn = kn_pool.tile([128, ncols], bf16, tag=tagbase, name=tagbase)
                for h in range(ncols // XCH):
                    h0 = h * XCH
                    x_lo = xin.tile([128, XCH], bf16, tag="xin", name="x_lo")
                    x_hi = xin.tile([128, XCH], bf16, tag="xin", name="x_hi")
                    nc.sync.dma_start(out=x_lo[:], in_=x_ext[0:128, h0 : h0 + XCH])
                    nc.sync.dma_start(out=x_hi[:], in_=x_ext[128:256, h0 : h0 + XCH])
                    norm_chunk(x_lo, x_hi, xn, h0, XCH)
                return xn

            qn = phase_a(xq_ext, QBLK, "qn")
            k1n = phase_a(f1_ext, N, "k1n")
            k2n = phase_a(f2_ext, N, "k2n")

            # preload the exp table set while phase A / early sims drain:
            # dummy exp ordered after the last ars so only 2 table loads total.
            edum = consts.tile([128, 1], f16, tag="edum", name="edum")
            ex0 = nc.scalar.activation(out=edum[:], in_=warm[:], func=AF.Exp)
            add_dep_helper(
                ex0.ins, last_ars.ins, sync=False,
                reason="order all ars (ars table) before exp table load",
            )

        with tc.tile_pool(name="sim_psum", bufs=2, space="PSUM") as sim_psum:

            def phase_b(kn, s):
                """row block of sim + softmax for one K map, streamed to out[s]."""
                for t in range(QBLK // 128):
                    lhsT = qn[:, t * 128 : (t + 1) * 128]
                    attn = attn_pool.tile([128, N], f16, tag="attn", name="attn")
                    stile = stat_pool.tile([128, 2], f32, tag="stile", name="stile")
                    e_chunks = []
                    for j in range(N // CH):
                        ps = sim_psum.tile([128, CH], f32, tag="sim", name="sim_ps")
                        for c in range(CH // 512):
                            csl = slice(j * CH + c * 512, j * CH + (c + 1) * 512)
                            nc.tensor.matmul(
                                ps[:, c * 512 : (c + 1) * 512],
                                lhsT,
                                kn[:, csl],
                                start=True,
                                stop=True,
                            )
                        e = e_pool.tile([128, CH], f16, tag="e", name="e")
                        nc.scalar.activation(
                            out=e[:],
                            in_=ps[:],
                            func=AF.Exp,
                            accum_out=stile[:, j : j + 1],
                        )
                        e_chunks.append(e)
                    ssum = stat_pool.tile([128, 1], f32, tag="ssum", name="ssum")
                    nc.vector.reduce_sum(ssum[:], stile[:], axis=mybir.AxisListType.X)
                    recip = stat_pool.tile([128, 1], f32, tag="recip", name="recip")
                    nc.vector.reciprocal(recip[:], ssum[:])
                    for j, e in enumerate(e_chunks):
                        nc.vector.tensor_scalar_mul(
                            attn[:, j * CH : (j + 1) * CH], e[:], recip[:]
                        )
                        nc.sync.dma_start(
                            out=out_ext[
                                s, t * 128 : (t + 1) * 128, j * CH : (j + 1) * CH
                            ],
                            in_=attn[:, j * CH : (j + 1) * CH],
                        )

            phase_b(k1n, 0)
            phase_b(k2n, 1)

    nc.compile()
    return nc


def _get_nc():
    if "nc" not in _cached:
        _cached["nc"] = _build()
    return _cached["nc"]


def _in_maps(fmap1, fmap2, dmap, Wqk):
    bf = ml_dtypes.bfloat16
    f1r = np.asarray(fmap1, dtype=np.float32).reshape(B, C, N)
    f2r = np.asarray(fmap2, dtype=np.float32).reshape(B, C, N)
    dqr = np.asarray(dmap, dtype=np.float32).reshape(B, C, N)
    wT = np.ascontiguousarray(np.asarray(Wqk, dtype=np.float32).T).astype(bf)

    in_maps = []
    for i in range(N_CORES):
        b, r = divmod(i, 4)
        in_maps.append(
            {
                "f1": np.ascontiguousarray(f1r[b]).astype(bf),
                "f2": np.ascontiguousarray(f2r[b]).astype(bf),
                "xq": np.ascontiguousarray(
                    dqr[b][:, r * QBLK : (r + 1) * QBLK]
                ).astype(bf),
                "wqkT": wT,
            }
        )
    return in_maps


def kernel(fmap1, fmap2, dmap, Wqk):
    from concourse.bass_utils import run_bass_kernel_spmd

    in_maps = _in_maps(fmap1, fmap2, dmap, Wqk)
    nc = _get_nc()
    res = run_bass_kernel_spmd(nc, in_maps, core_ids=list(range(N_CORES)))
    _cached["last_result"] = res

    attn1 = np.empty((B, 1, N, N), dtype=np.float32)
    attn2 = np.empty((B, 1, N, N), dtype=np.float32)
    for i in range(N_CORES):
        b, r = divmod(i, 4)
        o = res.results[i]["out"]
        attn1[b, 0, r * QBLK : (r + 1) * QBLK, :] = o[0].astype(np.float32)
        attn2[b, 0, r * QBLK : (r + 1) * QBLK, :] = o[1].astype(np.float32)
    return (attn1, attn2)
